# revision 1
# baseline (speedup 1.0000x reference)
"""LoRA 4-bit linear layer for Trainium2, 8 NeuronCores.

Reference computation (per problem nn_LoRALayer4bit):
    W    = bf16(dequant4bit(q_weight, scales))          # [4096, 4096]
    out  = x @ W.T + 2.0 * ((x @ lora_A.T) @ lora_B.T)  # x: [4, 2048, 4096] bf16

Strategy:
  - Host folds the LoRA low-rank update into the dequantized weight:
        W_eff = bf16(f32(W) + 2.0 * lora_B @ lora_A)
    (differs from the two-path reference by <= 1-2 bf16 ulps on the output).
  - Row-parallel over the 8 cores: each core computes 1024 tokens x full
    4096 out-features (34.4 GFLOP/core).  No collectives; host concatenates.
  - Host pre-transposes each x shard to K-on-partitions layout, packed per
    128-token chunk in SBUF destination order, so each chunk is ONE
    contiguous 1MB full-line-rate DMA and the first PSUM group is gated on
    just wt-block-0 (4.2MB) + 1MB of x.
  - Device kernel: pure bf16 matmul; x shard resident in SBUF, weight
    blocks streamed double-buffered; 32 K-tiles accumulate into one PSUM
    bank per [128 x 512] output tile.
  - Warm-up matmuls on zeroed scratch keep the PE busy during the initial
    DMA fill so the HAM clock gate releases to 2.4GHz before the real
    matmuls start (a cold PE at 1.2GHz doubles matmul time).
"""

import numpy as np
import ml_dtypes

BF16 = ml_dtypes.bfloat16

IN_F = 4096
OUT_F = 4096
R = 16
SCALING = 2.0
BLK = 64
BATCH = 4
SEQ = 2048
N_CORES = 8

M_TOT = BATCH * SEQ            # 8192 tokens
M_PER = M_TOT // N_CORES       # 1024 tokens per core
KT = IN_F // 128               # 32 contraction tiles
NB = OUT_F // 512              # 8 out-feature blocks
MT = M_PER // 128              # 8 token sub-tiles per core

_CACHE = {}


def _build_nc():
    """Build + compile the single-core SPMD Bass program (cached)."""
    import concourse.bacc as bacc
    import concourse.tile as tile
    from concourse import mybir

    nc = bacc.Bacc(
        "TRN2", target_bir_lowering=False, debug=False, enable_asserts=False
    )

    # xt[m, p, k*128+c] = x_shard[m*128 + c, k*128 + p]  (dest-order packed)
    # wt[nb, k, p, c]   = W_eff[nb*512 + c, k*128 + p]
    # out[nb, m, p, c]  = out_shard[m*128 + p, nb*512 + c]
    xt_d = nc.dram_tensor(
        "xt", [MT, 128, KT * 128], mybir.dt.bfloat16, kind="ExternalInput"
    )
    wt_d = nc.dram_tensor(
        "wt", [NB, KT, 128, 512], mybir.dt.bfloat16, kind="ExternalInput"
    )
    out_d = nc.dram_tensor(
        "out", [NB, MT, 128, 512], mybir.dt.bfloat16, kind="ExternalOutput"
    )

    N_WARM = 96

    with tile.TileContext(nc) as tc:
        with (
            tc.tile_pool(name="xp", bufs=MT) as xp,
            tc.tile_pool(name="wp", bufs=2 * KT) as wp,
            tc.tile_pool(name="op", bufs=4) as op,
            tc.tile_pool(name="pp", bufs=6, space="PSUM") as pp,
            tc.tile_pool(name="wu", bufs=3) as wu,
        ):
            # Warm-up: dummy matmuls on (uninitialized) scratch, alternating
            # between two PSUM banks so they stream back-to-back.  Their
            # results are never read; they only keep the PE busy so the HAM
            # clock gate releases while the first DMAs land.
            wa = wu.tile([128, 128], mybir.dt.bfloat16, name="wa", tag="wa")
            wr = wu.tile([128, 512], mybir.dt.bfloat16, name="wr", tag="wr")
            nc.vector.memset(wa[:], 0.0)
            nc.vector.memset(wr[:], 0.0)
            wps0 = pp.tile(
                [128, 512], mybir.dt.float32, name="wps0", tag="wu0", bufs=1
            )
            wps1 = pp.tile(
                [128, 512], mybir.dt.float32, name="wps1", tag="wu1", bufs=1
            )
            for i in range(N_WARM):
                nc.tensor.matmul(
                    (wps0 if i % 2 == 0 else wps1)[:],
                    wa[:], wr[:], start=True, stop=True,
                )

            # First x m-chunk (one contiguous 1MB DMA) + first weight block.
            # The remaining chunks are staggered between the first block's
            # compute groups to smooth the initial HBM burst.
            xms = [None] * MT
            xm0 = xp.tile(
                [128, KT * 128], mybir.dt.bfloat16, name="xm0", tag="xm"
            )
            nc.sync.dma_start(xm0[:], xt_d[0])
            xms[0] = xm0
            wts0 = []
            for k in range(KT):
                wtile = wp.tile(
                    [128, 512], mybir.dt.bfloat16, name=f"w0_{k}", tag="wt"
                )
                nc.sync.dma_start(wtile[:], wt_d[0, k])
                wts0.append(wtile)

            for nb in range(NB):
                if nb == 0:
                    wts = wts0
                else:
                    # Streams during block nb-1's compute (wp holds 2 blocks).
                    wts = []
                    for k in range(KT):
                        wtile = wp.tile(
                            [128, 512], mybir.dt.bfloat16, name=f"w{nb}_{k}", tag="wt"
                        )
                        nc.sync.dma_start(wtile[:], wt_d[nb, k])
                        wts.append(wtile)

                for m in range(MT):
                    if nb == 0 and m + 1 < MT:
                        xm = xp.tile(
                            [128, KT * 128],
                            mybir.dt.bfloat16,
                            name=f"xm{m + 1}",
                            tag="xm",
                        )
                        nc.sync.dma_start(xm[:], xt_d[m + 1])
                        xms[m + 1] = xm
                    ps = pp.tile(
                        [128, 512], mybir.dt.float32, name=f"ps{nb}_{m}", tag="ps"
                    )
                    for k in range(KT):
                        nc.tensor.matmul(
                            ps[:],
                            xms[m][:, k * 128 : (k + 1) * 128],
                            wts[k][:],
                            start=(k == 0),
                            stop=(k == KT - 1),
                        )
                    ot = op.tile(
                        [128, 512], mybir.dt.bfloat16, name=f"o{nb}_{m}", tag="ot"
                    )
                    nc.vector.tensor_copy(ot[:], ps[:])
                    nc.sync.dma_start(out_d[nb, m], ot[:])

    nc.compile()
    return nc


def _prep_weights(q_weight, scales, lora_A, lora_B):
    q = np.asarray(q_weight)
    s = np.asarray(scales, dtype=np.float32)
    # Exactly the reference dequant: per-64-block scale, rounded to bf16.
    W = (
        (q.astype(np.float32).reshape(OUT_F, IN_F // BLK, BLK) * s[:, :, None])
        .reshape(OUT_F, IN_F)
        .astype(BF16)
    )
    BA = np.asarray(lora_B, dtype=np.float32) @ np.asarray(lora_A, dtype=np.float32)
    W_eff = (W.astype(np.float32) + SCALING * BA).astype(BF16)
    # [nb, k, p, c] = W_eff[nb*512+c, k*128+p]
    wt = np.ascontiguousarray(
        W_eff.reshape(NB, 512, KT, 128).transpose(0, 2, 3, 1)
    )
    return wt


def kernel(x, q_weight, scales, lora_A, lora_B):
    from concourse.bass_utils import run_bass_kernel_spmd

    if "nc" not in _CACHE:
        _CACHE["nc"] = _build_nc()
    nc = _CACHE["nc"]

    wt = _prep_weights(q_weight, scales, lora_A, lora_B)

    xf = np.ascontiguousarray(np.asarray(x)).reshape(M_TOT, IN_F)
    in_maps = []
    for c in range(N_CORES):
        xs = xf[c * M_PER : (c + 1) * M_PER]          # [1024, 4096]
        # [m, p, k, c2] = xs[m*128+c2, k*128+p]
        xt = np.ascontiguousarray(
            xs.reshape(MT, 128, KT, 128).transpose(0, 3, 2, 1)
        ).reshape(MT, 128, KT * 128)
        in_maps.append({"xt": xt, "wt": wt})

    res = run_bass_kernel_spmd(nc, in_maps, core_ids=list(range(N_CORES)))
    _CACHE["last_results"] = res

    shards = []
    for c in range(N_CORES):
        o = np.asarray(res.results[c]["out"])          # [NB, MT, 128, 512]
        shards.append(o.transpose(1, 2, 0, 3).reshape(M_PER, OUT_F))
    out = np.concatenate(shards, axis=0).reshape(BATCH, SEQ, OUT_F)
    return out.astype(BF16)



# revision 2
# speedup vs baseline: 1.0066x; 1.0066x over previous
"""LoRA 4-bit linear layer for Trainium2, 8 NeuronCores.

Reference computation (per problem nn_LoRALayer4bit):
    W    = bf16(dequant4bit(q_weight, scales))          # [4096, 4096]
    out  = x @ W.T + 2.0 * ((x @ lora_A.T) @ lora_B.T)  # x: [4, 2048, 4096] bf16

Strategy:
  - Host folds the LoRA low-rank update into the dequantized weight:
        W_eff = bf16(f32(W) + 2.0 * lora_B @ lora_A)
  - Row-parallel over the 8 cores: each core computes 1024 tokens x full
    4096 out-features.  No collectives; host concatenates.
  - Host pre-transposes each x shard to K-on-partitions layout (one
    contiguous 1MB DMA per 128-token chunk) and packs W per 512-feature
    block as [128, KT, 512] so each block is ONE contiguous 4MB DMA
    (32KB per partition line).
  - Device kernel: pure bf16 matmul; x shard resident in SBUF, weight
    blocks streamed double-buffered; 32 K-tiles accumulate into one PSUM
    bank per [128 x 512] output tile.
  - Short warm-up matmul train keeps the PE busy (and its clock ramped)
    exactly until the first weight block lands.
"""

import numpy as np
import ml_dtypes

BF16 = ml_dtypes.bfloat16

IN_F = 4096
OUT_F = 4096
R = 16
SCALING = 2.0
BLK = 64
BATCH = 4
SEQ = 2048
N_CORES = 8

M_TOT = BATCH * SEQ            # 8192 tokens
M_PER = M_TOT // N_CORES       # 1024 tokens per core
KT = IN_F // 128               # 32 contraction tiles
NB = OUT_F // 512              # 8 out-feature blocks
MT = M_PER // 128              # 8 token sub-tiles per core

_CACHE = {}


def _build_nc():
    """Build + compile the single-core SPMD Bass program (cached)."""
    import concourse.bacc as bacc
    import concourse.tile as tile
    from concourse import mybir

    nc = bacc.Bacc(
        "TRN2", target_bir_lowering=False, debug=False, enable_asserts=False
    )

    # xt[m, p, k*128+c] = x_shard[m*128 + c, k*128 + p]  (dest-order packed)
    # wt[nb, p, k, c]   = W_eff[nb*512 + c, k*128 + p]
    # out[nb, m, p, c]  = out_shard[m*128 + p, nb*512 + c]
    xt_d = nc.dram_tensor(
        "xt", [MT, 128, KT * 128], mybir.dt.bfloat16, kind="ExternalInput"
    )
    wt_d = nc.dram_tensor(
        "wt", [NB, 128, KT, 512], mybir.dt.bfloat16, kind="ExternalInput"
    )
    out_d = nc.dram_tensor(
        "out", [NB, MT, 128, 512], mybir.dt.bfloat16, kind="ExternalOutput"
    )

    N_WARM = 68

    with tile.TileContext(nc) as tc:
        with (
            tc.tile_pool(name="xp", bufs=MT) as xp,
            tc.tile_pool(name="wp", bufs=2) as wp,
            tc.tile_pool(name="op", bufs=4) as op,
            tc.tile_pool(name="pp", bufs=6, space="PSUM") as pp,
            tc.tile_pool(name="wu", bufs=3) as wu,
        ):
            # Warm-up: dummy matmuls on zeroed scratch, alternating between
            # two PSUM banks so they stream back-to-back.  They keep the PE
            # busy (and its clock ramped) while the first DMAs land.
            wa = wu.tile([128, 128], mybir.dt.bfloat16, name="wa", tag="wa")
            wr = wu.tile([128, 512], mybir.dt.bfloat16, name="wr", tag="wr")
            nc.vector.memset(wa[:], 0.0)
            nc.vector.memset(wr[:], 0.0)
            wps0 = pp.tile(
                [128, 512], mybir.dt.float32, name="wps0", tag="wu0", bufs=1
            )
            wps1 = pp.tile(
                [128, 512], mybir.dt.float32, name="wps1", tag="wu1", bufs=1
            )

            # First x m-chunk + first weight block: issued before the warmup
            # so their transfers run under it.  One DMA each.
            xms = [None] * MT
            xm0 = xp.tile(
                [128, KT * 128], mybir.dt.bfloat16, name="xm0", tag="xm"
            )
            nc.sync.dma_start(xm0[:], xt_d[0])
            xms[0] = xm0
            wts = [None, None]
            w0 = wp.tile([128, KT, 512], mybir.dt.bfloat16, name="wb0", tag="wt")
            nc.sync.dma_start(w0[:], wt_d[0])
            wts[0] = w0

            for i in range(N_WARM):
                nc.tensor.matmul(
                    (wps0 if i % 2 == 0 else wps1)[:],
                    wa[:], wr[:], start=True, stop=True,
                )

            for nb in range(NB):
                if nb + 1 < NB:
                    # Next block streams during this block's compute.
                    wnxt = wp.tile(
                        [128, KT, 512], mybir.dt.bfloat16,
                        name=f"wb{nb + 1}", tag="wt",
                    )
                    nc.sync.dma_start(wnxt[:], wt_d[nb + 1])
                    wts[(nb + 1) % 2] = wnxt
                wb = wts[nb % 2]

                for m in range(MT):
                    if nb == 0 and m + 1 < MT:
                        xm = xp.tile(
                            [128, KT * 128],
                            mybir.dt.bfloat16,
                            name=f"xm{m + 1}",
                            tag="xm",
                        )
                        nc.sync.dma_start(xm[:], xt_d[m + 1])
                        xms[m + 1] = xm
                    ps = pp.tile(
                        [128, 512], mybir.dt.float32, name=f"ps{nb}_{m}", tag="ps"
                    )
                    for k in range(KT):
                        nc.tensor.matmul(
                            ps[:],
                            xms[m][:, k * 128 : (k + 1) * 128],
                            wb[:, k, :],
                            start=(k == 0),
                            stop=(k == KT - 1),
                        )
                    ot = op.tile(
                        [128, 512], mybir.dt.bfloat16, name=f"o{nb}_{m}", tag="ot"
                    )
                    nc.vector.tensor_copy(ot[:], ps[:])
                    nc.sync.dma_start(out_d[nb, m], ot[:])

    nc.compile()
    return nc


def _prep_weights(q_weight, scales, lora_A, lora_B):
    q = np.asarray(q_weight)
    s = np.asarray(scales, dtype=np.float32)
    # Exactly the reference dequant: per-64-block scale, rounded to bf16.
    W = (
        (q.astype(np.float32).reshape(OUT_F, IN_F // BLK, BLK) * s[:, :, None])
        .reshape(OUT_F, IN_F)
        .astype(BF16)
    )
    BA = np.asarray(lora_B, dtype=np.float32) @ np.asarray(lora_A, dtype=np.float32)
    W_eff = (W.astype(np.float32) + SCALING * BA).astype(BF16)
    # [nb, p, k, c] = W_eff[nb*512+c, k*128+p]
    wt = np.ascontiguousarray(
        W_eff.reshape(NB, 512, KT, 128).transpose(0, 3, 2, 1)
    )
    return wt


def kernel(x, q_weight, scales, lora_A, lora_B):
    from concourse.bass_utils import run_bass_kernel_spmd

    if "nc" not in _CACHE:
        _CACHE["nc"] = _build_nc()
    nc = _CACHE["nc"]

    wt = _prep_weights(q_weight, scales, lora_A, lora_B)

    xf = np.ascontiguousarray(np.asarray(x)).reshape(M_TOT, IN_F)
    in_maps = []
    for c in range(N_CORES):
        xs = xf[c * M_PER : (c + 1) * M_PER]          # [1024, 4096]
        # [m, p, k, c2] = xs[m*128+c2, k*128+p]
        xt = np.ascontiguousarray(
            xs.reshape(MT, 128, KT, 128).transpose(0, 3, 2, 1)
        ).reshape(MT, 128, KT * 128)
        in_maps.append({"xt": xt, "wt": wt})

    res = run_bass_kernel_spmd(nc, in_maps, core_ids=list(range(N_CORES)))
    _CACHE["last_results"] = res

    shards = []
    for c in range(N_CORES):
        o = np.asarray(res.results[c]["out"])          # [NB, MT, 128, 512]
        shards.append(o.transpose(1, 2, 0, 3).reshape(M_PER, OUT_F))
    out = np.concatenate(shards, axis=0).reshape(BATCH, SEQ, OUT_F)
    return out.astype(BF16)


# revision 3
# speedup vs baseline: 1.2270x; 1.2189x over previous
"""LoRA 4-bit linear layer for Trainium2, 8 NeuronCores.

Reference computation (per problem nn_LoRALayer4bit):
    W    = bf16(dequant4bit(q_weight, scales))          # [4096, 4096]
    out  = x @ W.T + 2.0 * ((x @ lora_A.T) @ lora_B.T)  # x: [4, 2048, 4096] bf16

Strategy:
  - Host folds the LoRA low-rank update into the dequantized weight:
        W_eff = bf16(f32(W) + 2.0 * lora_B @ lora_A)
  - Row-parallel over the 8 cores: each core computes 1024 tokens x full
    4096 out-features.  No collectives; host concatenates.
  - Host pre-transposes each x shard to K-on-partitions layout (one
    contiguous 1MB DMA per 128-token chunk) and packs W per 512-feature
    block as [128, KT, 512] so each block is ONE contiguous 4MB DMA
    (32KB per partition line).
  - Device kernel: pure bf16 matmul; x shard resident in SBUF, weight
    blocks streamed double-buffered; 32 K-tiles accumulate into one PSUM
    bank per [128 x 512] output tile.
  - Short warm-up matmul train keeps the PE busy (and its clock ramped)
    exactly until the first weight block lands.
"""

import numpy as np
import ml_dtypes

BF16 = ml_dtypes.bfloat16

IN_F = 4096
OUT_F = 4096
R = 16
SCALING = 2.0
BLK = 64
BATCH = 4
SEQ = 2048
N_CORES = 8

M_TOT = BATCH * SEQ            # 8192 tokens
M_PER = M_TOT // N_CORES       # 1024 tokens per core
KT = IN_F // 128               # 32 contraction tiles
NB = OUT_F // 512              # 8 out-feature blocks
MT = M_PER // 128              # 8 token sub-tiles per core

_CACHE = {}


def _build_nc():
    """Build + compile the single-core SPMD Bass program (cached)."""
    import concourse.bacc as bacc
    import concourse.tile as tile
    from concourse import mybir

    nc = bacc.Bacc(
        "TRN2", target_bir_lowering=False, debug=False, enable_asserts=False
    )

    # xt[m, p, k*128+c] = x_shard[m*128 + c, k*128 + p]  (dest-order packed)
    # wt[nb, p, k, c]   = W_eff[nb*512 + c, k*128 + p]
    # out[nb, m, p, c]  = out_shard[m*128 + p, nb*512 + c]
    xt_d = nc.dram_tensor(
        "xt", [MT, 128, KT * 128], mybir.dt.bfloat16, kind="ExternalInput"
    )
    wt_d = nc.dram_tensor(
        "wt", [NB, 128, KT, 512], mybir.dt.bfloat16, kind="ExternalInput"
    )
    out_d = nc.dram_tensor(
        "out", [NB, MT, 128, 512], mybir.dt.bfloat16, kind="ExternalOutput"
    )

    N_WARM = 16
    W0_SPLIT = 8                   # sub-DMAs for block 0 (compute starts early)

    with tile.TileContext(nc) as tc:
        with (
            tc.tile_pool(name="xp", bufs=MT) as xp,
            tc.tile_pool(name="wp", bufs=2) as wp,
            tc.tile_pool(name="op", bufs=4) as op,
            tc.tile_pool(name="pp", bufs=6, space="PSUM") as pp,
            tc.tile_pool(name="wu", bufs=3) as wu,
        ):
            # Warm-up: dummy matmuls on zeroed scratch, alternating between
            # two PSUM banks so they stream back-to-back.  They keep the PE
            # busy (and its clock ramped) while the first DMAs land.
            wa = wu.tile([128, 128], mybir.dt.bfloat16, name="wa", tag="wa")
            wr = wu.tile([128, 512], mybir.dt.bfloat16, name="wr", tag="wr")
            nc.vector.memset(wa[:], 0.0)
            nc.vector.memset(wr[:], 0.0)
            wps0 = pp.tile(
                [128, 512], mybir.dt.float32, name="wps0", tag="wu0", bufs=1
            )
            wps1 = pp.tile(
                [128, 512], mybir.dt.float32, name="wps1", tag="wu1", bufs=1
            )

            # x chunk 0 + weight block 0 (in 8 sub-slices so the first chain
            # can start as soon as the first K-slices land), then the
            # remaining x chunks.  All issued before the warmup so their
            # transfers run under it / under block-0 compute.
            xms = [None] * MT
            xm0 = xp.tile(
                [128, KT * 128], mybir.dt.bfloat16, name="xm0", tag="xm"
            )
            nc.sync.dma_start(xm0[:], xt_d[0])
            xms[0] = xm0
            wts = [None, None]
            w0 = wp.tile([128, KT, 512], mybir.dt.bfloat16, name="wb0", tag="wt")
            kg = KT // W0_SPLIT
            for g in range(W0_SPLIT):
                nc.sync.dma_start(
                    w0[:, g * kg : (g + 1) * kg, :],
                    wt_d[0, :, g * kg : (g + 1) * kg, :],
                )
            wts[0] = w0
            for m in range(1, MT):
                xm = xp.tile(
                    [128, KT * 128], mybir.dt.bfloat16, name=f"xm{m}", tag="xm"
                )
                nc.sync.dma_start(xm[:], xt_d[m])
                xms[m] = xm

            for i in range(N_WARM):
                nc.tensor.matmul(
                    (wps0 if i % 2 == 0 else wps1)[:],
                    wa[:], wr[:], start=True, stop=True,
                )

            for nb in range(NB):
                if nb + 1 < NB:
                    # Next block streams during this block's compute.
                    wnxt = wp.tile(
                        [128, KT, 512], mybir.dt.bfloat16,
                        name=f"wb{nb + 1}", tag="wt",
                    )
                    nc.sync.dma_start(wnxt[:], wt_d[nb + 1])
                    wts[(nb + 1) % 2] = wnxt
                wb = wts[nb % 2]

                for m in range(MT):
                    ps = pp.tile(
                        [128, 512], mybir.dt.float32, name=f"ps{nb}_{m}", tag="ps"
                    )
                    for k in range(KT):
                        nc.tensor.matmul(
                            ps[:],
                            xms[m][:, k * 128 : (k + 1) * 128],
                            wb[:, k, :],
                            start=(k == 0),
                            stop=(k == KT - 1),
                        )
                    ot = op.tile(
                        [128, 512], mybir.dt.bfloat16, name=f"o{nb}_{m}", tag="ot"
                    )
                    nc.vector.tensor_copy(ot[:], ps[:])
                    nc.sync.dma_start(out_d[nb, m], ot[:])

    nc.compile()
    return nc


def _prep_weights(q_weight, scales, lora_A, lora_B):
    q = np.asarray(q_weight)
    s = np.asarray(scales, dtype=np.float32)
    # Exactly the reference dequant: per-64-block scale, rounded to bf16.
    W = (
        (q.astype(np.float32).reshape(OUT_F, IN_F // BLK, BLK) * s[:, :, None])
        .reshape(OUT_F, IN_F)
        .astype(BF16)
    )
    BA = np.asarray(lora_B, dtype=np.float32) @ np.asarray(lora_A, dtype=np.float32)
    W_eff = (W.astype(np.float32) + SCALING * BA).astype(BF16)
    # [nb, p, k, c] = W_eff[nb*512+c, k*128+p]
    wt = np.ascontiguousarray(
        W_eff.reshape(NB, 512, KT, 128).transpose(0, 3, 2, 1)
    )
    return wt


def kernel(x, q_weight, scales, lora_A, lora_B):
    from concourse.bass_utils import run_bass_kernel_spmd

    if "nc" not in _CACHE:
        _CACHE["nc"] = _build_nc()
    nc = _CACHE["nc"]

    wt = _prep_weights(q_weight, scales, lora_A, lora_B)

    xf = np.ascontiguousarray(np.asarray(x)).reshape(M_TOT, IN_F)
    in_maps = []
    for c in range(N_CORES):
        xs = xf[c * M_PER : (c + 1) * M_PER]          # [1024, 4096]
        # [m, p, k, c2] = xs[m*128+c2, k*128+p]
        xt = np.ascontiguousarray(
            xs.reshape(MT, 128, KT, 128).transpose(0, 3, 2, 1)
        ).reshape(MT, 128, KT * 128)
        in_maps.append({"xt": xt, "wt": wt})

    res = run_bass_kernel_spmd(nc, in_maps, core_ids=list(range(N_CORES)))
    _CACHE["last_results"] = res

    shards = []
    for c in range(N_CORES):
        o = np.asarray(res.results[c]["out"])          # [NB, MT, 128, 512]
        shards.append(o.transpose(1, 2, 0, 3).reshape(M_PER, OUT_F))
    out = np.concatenate(shards, axis=0).reshape(BATCH, SEQ, OUT_F)
    return out.astype(BF16)
